# revision 9
# baseline (speedup 1.0000x reference)
"""Trainium2 Bass kernel for the KnowledgeGraphEmbedding loss.

Computes, for P=1024 relations sharded 128-per-core across 8 NeuronCores:
    li = Lp_w[p] @ wi          (wi = tag_rep[tag1_idx])
    rj = Rp_w[p] @ wj          (wj = tag_rep[tag2_idx])
    dist[p] = sum_h (li - rj)^2
    out = [dist*rel, dist*(1-rel), rel, 1-rel]   (rel in {0,1})

Strategy (memory-bound; ~23MB of fp8 weights streamed per core):
  - Weights quantized to fp8 e4m3 (4x fewer bytes than f32). Host packs,
    per core, X[p, h, 0:600] = [Lp_w[p,h,:], Rp_w[p,h,:]] * s_w transposed
    to column-major X_T[e, p*300+h], and v = [wi; -wj] * s_v split into
    v_hi + v_lo (fp8 residual) so the effective v error is ~2^-8.
    Overall rel err ~7.7e-3 vs the 2e-2 gate.
  - TensorE contracts with DoubleRow perf mode (2 fp8 weights/cell, ~2x
    moving throughput): per relation, 6 matmuls (3 contraction chunks of
    K_logical={240,240,120} x {v_hi, v_lo}) accumulate diff[p, 0:300]
    into one PSUM row at partition 0.
  - Contraction rows are pre-interleaved on host into the DoubleRow
    [K_phys, 2, N] layout: xd_g[k, i, col] = X_T[chunk_g_base + i*Kp + k].
  - DMA: 3 mega-DMAs per 16-relation round (one per chunk tensor), spread
    over SP-HWDGE, ACT-HWDGE and gpsimd-SWDGE rings to saturate HBM.
  - Drain: one fused square+reduce per relation: ACT activation(Square,
    accum_out) for 9/16, DVE (copy + scalar_tensor_tensor accum) for 7/16.
  - Bins at the end are a few [1,128] DVE ops + one DMA.
"""

from contextlib import ExitStack

import numpy as np

N_CORES = 8
P_TOTAL = 1024
H = 300
E = 300
P_LOC = P_TOTAL // N_CORES   # 128 relations per core
KK = 600                     # contraction length (L and R concatenated)
# DoubleRow chunking: K_logical = {240, 240, 120}, K_phys = {120, 120, 60}
KPHYS = (120, 120, 60)
KBASE = (0, 240, 480)
ROUND_P = 16                 # relations per DMA round
NROUND = P_LOC // ROUND_P    # 8
CSUP = ROUND_P * H           # 4800 columns per round

F8_TARGET_FRAC = 0.6         # amax target as fraction of fp8 max

TRACE = False
LAST_RESULT = None

_CACHE: dict = {}


def _build_nc():
    import concourse.bacc as bacc
    import concourse.mybir as mybir
    import concourse.tile as tile

    f32 = mybir.dt.float32
    f8 = mybir.dt.float8e4
    DR = mybir.MatmulPerfMode.DoubleRow

    nc = bacc.Bacc("TRN2", debug=False)

    # xd_g[k, i*38400 + col] = X_T[KBASE[g] + i*KPHYS[g] + k, col]
    xds = [
        nc.dram_tensor(f"xd{g}", [KPHYS[g], 2 * P_LOC * H], f8,
                       kind="ExternalInput").ap()
        for g in range(3)
    ]
    # wv[k, i*16 + 3*half + g] = v_{half}[KBASE[g] + i*KPHYS[g] + k]
    # (DoubleRow LDWEIGHTS needs the i-pair at a step%16==0 stride)
    wv = nc.dram_tensor("wv", [120, 32], f8, kind="ExternalInput").ap()
    # rm: [rel*k2inv(128), (1-rel)*k2inv(128), rel(128), 1-rel(128), 0.0]
    rm = nc.dram_tensor("rm", [1, 513], f32, kind="ExternalInput").ap()
    out = nc.dram_tensor("out", [4, P_LOC], f32, kind="ExternalOutput").ap()

    with tile.TileContext(nc) as tc, ExitStack() as ctx:
        const_pool = ctx.enter_context(tc.tile_pool(name="const", bufs=1))
        data_pool = ctx.enter_context(tc.tile_pool(name="data", bufs=2))
        psum_pool = ctx.enter_context(
            tc.tile_pool(name="psum", bufs=8, space="PSUM")
        )
        scr_pool = ctx.enter_context(tc.tile_pool(name="scr", bufs=2))

        v_sb = const_pool.tile([120, 32], f8)
        nc.sync.dma_start(v_sb[:], wv[:])
        rm_sb = const_pool.tile([1, 513], f32)
        nc.sync.dma_start(rm_sb[:], rm[:])
        dist_sb = const_pool.tile([1, P_LOC], f32)
        outp = const_pool.tile([1, 4 * P_LOC], f32)

        queues = (nc.sync, nc.scalar, nc.gpsimd)

        for r in range(NROUND):
            dts = []
            for g in range(3):
                kp = KPHYS[g]
                dt_ = data_pool.tile([kp, 2 * CSUP], f8, name=f"dt{g}")
                src = xds[g].rearrange("k (i c) -> k i c", i=2)[
                    :, :, r * CSUP : (r + 1) * CSUP
                ]
                dst = dt_.rearrange("k (i c) -> k i c", i=2)
                queues[g].dma_start(dst, src)
                dts.append(dt_)

            for pi in range(ROUND_P):
                pt = psum_pool.tile([1, H], f32, name="pt")
                row = pt[0:1, :]
                n_mm = 0
                for half in range(2):          # 0: v_hi, 1: v_lo
                    for g in range(3):
                        kp = KPHYS[g]
                        rhs = dts[g].rearrange("k (i c) -> k i c", i=2)[
                            :, :, pi * H : (pi + 1) * H
                        ]
                        c_w = 3 * half + g
                        lhsT = v_sb.rearrange("k (i c) -> k i c", i=2)[
                            0:kp, :, c_w : c_w + 1
                        ]
                        nc.tensor.matmul(
                            out=row,
                            lhsT=lhsT,
                            rhs=rhs,
                            start=(n_mm == 0),
                            stop=(n_mm == 5),
                            perf_mode=DR,
                        )
                        n_mm += 1

                p = r * ROUND_P + pi
                col = dist_sb[0:1, p : p + 1]
                if pi % 16 < 9:
                    nc.scalar.activation(
                        row,
                        row,
                        mybir.ActivationFunctionType.Square,
                        bias=rm_sb[0:1, 512:513],
                        scale=1.0,
                        accum_out=col,
                    )
                else:
                    scr = scr_pool.tile([1, H], f32, name="scr")
                    srow = scr[0:1, :]
                    nc.vector.tensor_copy(srow, row)
                    nc.vector.scalar_tensor_tensor(
                        out=srow,
                        in0=srow,
                        scalar=1.0,
                        in1=srow,
                        op0=mybir.AluOpType.mult,
                        op1=mybir.AluOpType.mult,
                        accum_out=col,
                    )

        d = dist_sb[0:1, :]
        nc.vector.tensor_mul(outp[:, 0:128], rm_sb[:, 0:128], d)
        nc.vector.tensor_mul(outp[:, 128:256], rm_sb[:, 128:256], d)
        nc.vector.tensor_copy(outp[:, 256:512], rm_sb[:, 256:512])
        nc.sync.dma_start(
            out[:, :], outp.rearrange("p (b q) -> p b q", b=4)
        )

    nc.compile()
    return nc


def kernel(tag_rep, Lp_w, Rp_w, relation, tag1_idx, tag2_idx):
    global LAST_RESULT
    import ml_dtypes
    from concourse.bass_utils import run_bass_kernel_spmd

    f8np = ml_dtypes.float8_e4m3
    f8max = float(ml_dtypes.finfo(f8np).max)

    if "nc" not in _CACHE:
        _CACHE["nc"] = _build_nc()
    nc = _CACHE["nc"]

    tag_rep = np.asarray(tag_rep)
    Lp_w = np.asarray(Lp_w, dtype=np.float32)
    Rp_w = np.asarray(Rp_w, dtype=np.float32)
    rel = np.asarray(relation).astype(np.float32)  # values in {0, 1}

    wi = tag_rep[int(tag1_idx)].astype(np.float32)
    wj = tag_rep[int(tag2_idx)].astype(np.float32)

    amax_w = max(np.abs(Lp_w).max(), np.abs(Rp_w).max())
    amax_v = max(np.abs(wi).max(), np.abs(wj).max())
    s_w = f8max * F8_TARGET_FRAC / float(amax_w)
    s_v = f8max * F8_TARGET_FRAC / float(amax_v)
    k2inv = 1.0 / (s_w * s_v) ** 2

    v = (np.concatenate([wi, -wj]) * s_v).astype(np.float32)   # [600]
    v_hi = v.astype(f8np)
    v_lo = (v - v_hi.astype(np.float32)).astype(f8np)

    wv_arr = np.zeros((120, 32), dtype=f8np)
    for half, vq in ((0, v_hi), (1, v_lo)):
        for g in range(3):
            kp = KPHYS[g]
            for i in range(2):
                wv_arr[0:kp, i * 16 + 3 * half + g] = vq[
                    KBASE[g] + i * kp : KBASE[g] + (i + 1) * kp
                ]

    in_maps = []
    for cidx in range(N_CORES):
        sl = slice(cidx * P_LOC, (cidx + 1) * P_LOC)
        # X_T[e, p*300+h] = concat(L, R over e)[p, h, e] * s_w, fp8
        xc = np.concatenate(
            [
                Lp_w[sl].transpose(2, 0, 1),   # [300, 128, 300]
                Rp_w[sl].transpose(2, 0, 1),
            ],
            axis=0,
        ).reshape(KK, P_LOC * H)
        xq = (xc * s_w).astype(f8np)

        core_map = {"wv": wv_arr}
        for g in range(3):
            kp = KPHYS[g]
            blk = xq[KBASE[g] : KBASE[g] + 2 * kp]        # [2*kp, 38400]
            core_map[f"xd{g}"] = np.ascontiguousarray(
                blk.reshape(2, kp, P_LOC * H).transpose(1, 0, 2)
            ).reshape(kp, 2 * P_LOC * H)

        rel_c = rel[sl]
        ra = np.zeros((1, 513), dtype=np.float32)
        ra[0, 0:128] = rel_c * k2inv
        ra[0, 128:256] = (1.0 - rel_c) * k2inv
        ra[0, 256:384] = rel_c
        ra[0, 384:512] = 1.0 - rel_c
        core_map["rm"] = ra
        in_maps.append(core_map)

    kw = {}
    if TRACE:
        kw = dict(trace=True, trace_cores=[0])
    res = run_bass_kernel_spmd(nc, in_maps, core_ids=list(range(N_CORES)), **kw)
    LAST_RESULT = res

    out_full = np.empty((4, P_TOTAL), dtype=np.float32)
    for cidx in range(N_CORES):
        out_full[:, cidx * P_LOC : (cidx + 1) * P_LOC] = res.results[cidx]["out"]
    return out_full


# revision 10
# speedup vs baseline: 1.0016x; 1.0016x over previous
"""Trainium2 Bass kernel for the KnowledgeGraphEmbedding loss.

Computes, for P=1024 relations sharded 128-per-core across 8 NeuronCores:
    li = Lp_w[p] @ wi          (wi = tag_rep[tag1_idx])
    rj = Rp_w[p] @ wj          (wj = tag_rep[tag2_idx])
    dist[p] = sum_h (li - rj)^2
    out = [dist*rel, dist*(1-rel), rel, 1-rel]   (rel in {0,1})

Strategy (memory-bound; ~23MB of fp8 weights streamed per core):
  - Weights quantized to fp8 e4m3 (4x fewer bytes than f32). Host packs,
    per core, X[p, h, 0:600] = [Lp_w[p,h,:], Rp_w[p,h,:]] * s_w transposed
    to column-major X_T[e, p*300+h], and v = [wi; -wj] * s_v split into
    v_hi + v_lo (fp8 residual) so the effective v error is ~2^-8.
    Overall rel err ~7.7e-3 vs the 2e-2 gate.
  - TensorE contracts with DoubleRow perf mode (2 fp8 weights/cell, ~2x
    moving throughput): per relation, 6 matmuls (3 contraction chunks of
    K_logical={240,240,120} x {v_hi, v_lo}) accumulate diff[p, 0:300]
    into one PSUM row at partition 0.
  - Contraction rows are pre-interleaved on host into the DoubleRow
    [K_phys, 2, N] layout: xd_g[k, i, col] = X_T[chunk_g_base + i*Kp + k].
  - DMA: 3 mega-DMAs per 16-relation round (one per chunk tensor), spread
    over SP-HWDGE, ACT-HWDGE and gpsimd-SWDGE rings to saturate HBM.
  - Drain: one fused square+reduce per relation: ACT activation(Square,
    accum_out) for 9/16, DVE (copy + scalar_tensor_tensor accum) for 7/16.
  - Bins at the end are a few [1,128] DVE ops + one DMA.
"""

from contextlib import ExitStack

import numpy as np

N_CORES = 8
P_TOTAL = 1024
H = 300
E = 300
P_LOC = P_TOTAL // N_CORES   # 128 relations per core
KK = 600                     # contraction length (L and R concatenated)
# DoubleRow chunking: K_logical = {240, 240, 120}, K_phys = {120, 120, 60}
KPHYS = (120, 120, 60)
KBASE = (0, 240, 480)
ROUND_P = 16                 # relations per DMA round
NROUND = P_LOC // ROUND_P    # 8
CSUP = ROUND_P * H           # 4800 columns per round

F8_TARGET_FRAC = 0.6         # amax target as fraction of fp8 max

TRACE = False
LAST_RESULT = None

_CACHE: dict = {}


def _build_nc():
    import concourse.bacc as bacc
    import concourse.mybir as mybir
    import concourse.tile as tile

    f32 = mybir.dt.float32
    f8 = mybir.dt.float8e4
    DR = mybir.MatmulPerfMode.DoubleRow

    nc = bacc.Bacc("TRN2", debug=False)

    # xd_g[k, col*2 + i] = X_T[KBASE[g] + i*KPHYS[g] + k, col]
    # (i-pairs adjacent in memory so a DoubleRow matmul reads both
    # elements of a pair from one SBUF line)
    xds = [
        nc.dram_tensor(f"xd{g}", [KPHYS[g], 2 * P_LOC * H], f8,
                       kind="ExternalInput").ap()
        for g in range(3)
    ]
    # wv[k, i*16 + 3*half + g] = v_{half}[KBASE[g] + i*KPHYS[g] + k]
    # (DoubleRow LDWEIGHTS needs the i-pair at a step%16==0 stride)
    wv = nc.dram_tensor("wv", [120, 32], f8, kind="ExternalInput").ap()
    # rm: [rel*k2inv(128), (1-rel)*k2inv(128), rel(128), 1-rel(128), 0.0]
    rm = nc.dram_tensor("rm", [1, 513], f32, kind="ExternalInput").ap()
    out = nc.dram_tensor("out", [4, P_LOC], f32, kind="ExternalOutput").ap()

    with tile.TileContext(nc) as tc, ExitStack() as ctx:
        const_pool = ctx.enter_context(tc.tile_pool(name="const", bufs=1))
        data_pool = ctx.enter_context(tc.tile_pool(name="data", bufs=2))
        psum_pool = ctx.enter_context(
            tc.tile_pool(name="psum", bufs=8, space="PSUM")
        )
        scr_pool = ctx.enter_context(tc.tile_pool(name="scr", bufs=2))

        v_sb = const_pool.tile([120, 32], f8)
        nc.sync.dma_start(v_sb[:], wv[:])
        rm_sb = const_pool.tile([1, 513], f32)
        nc.sync.dma_start(rm_sb[:], rm[:])
        dist_sb = const_pool.tile([1, P_LOC], f32)
        outp = const_pool.tile([1, 4 * P_LOC], f32)

        queues = (nc.sync, nc.scalar, nc.gpsimd)

        for r in range(NROUND):
            dts = []
            for g in range(3):
                kp = KPHYS[g]
                dt_ = data_pool.tile([kp, 2 * CSUP], f8, name=f"dt{g}")
                queues[g].dma_start(
                    dt_[:], xds[g][:, 2 * r * CSUP : 2 * (r + 1) * CSUP]
                )
                dts.append(dt_)

            for pi in range(ROUND_P):
                pt = psum_pool.tile([1, H], f32, name="pt")
                row = pt[0:1, :]
                n_mm = 0
                for half in range(2):          # 0: v_hi, 1: v_lo
                    for g in range(3):
                        kp = KPHYS[g]
                        rhs = dts[g].rearrange("k (c i) -> k i c", i=2)[
                            :, :, pi * H : (pi + 1) * H
                        ]
                        c_w = 3 * half + g
                        lhsT = v_sb.rearrange("k (i c) -> k i c", i=2)[
                            0:kp, :, c_w : c_w + 1
                        ]
                        nc.tensor.matmul(
                            out=row,
                            lhsT=lhsT,
                            rhs=rhs,
                            start=(n_mm == 0),
                            stop=(n_mm == 5),
                            perf_mode=DR,
                        )
                        n_mm += 1

                p = r * ROUND_P + pi
                col = dist_sb[0:1, p : p + 1]
                if pi % 16 < 9:
                    nc.scalar.activation(
                        row,
                        row,
                        mybir.ActivationFunctionType.Square,
                        bias=rm_sb[0:1, 512:513],
                        scale=1.0,
                        accum_out=col,
                    )
                else:
                    scr = scr_pool.tile([1, H], f32, name="scr")
                    srow = scr[0:1, :]
                    nc.vector.tensor_copy(srow, row)
                    nc.vector.scalar_tensor_tensor(
                        out=srow,
                        in0=srow,
                        scalar=1.0,
                        in1=srow,
                        op0=mybir.AluOpType.mult,
                        op1=mybir.AluOpType.mult,
                        accum_out=col,
                    )

        d = dist_sb[0:1, :]
        nc.vector.tensor_mul(outp[:, 0:128], rm_sb[:, 0:128], d)
        nc.vector.tensor_mul(outp[:, 128:256], rm_sb[:, 128:256], d)
        nc.vector.tensor_copy(outp[:, 256:512], rm_sb[:, 256:512])
        nc.sync.dma_start(
            out[:, :], outp.rearrange("p (b q) -> p b q", b=4)
        )

    nc.compile()
    return nc


def kernel(tag_rep, Lp_w, Rp_w, relation, tag1_idx, tag2_idx):
    global LAST_RESULT
    import ml_dtypes
    from concourse.bass_utils import run_bass_kernel_spmd

    f8np = ml_dtypes.float8_e4m3
    f8max = float(ml_dtypes.finfo(f8np).max)

    if "nc" not in _CACHE:
        _CACHE["nc"] = _build_nc()
    nc = _CACHE["nc"]

    tag_rep = np.asarray(tag_rep)
    Lp_w = np.asarray(Lp_w, dtype=np.float32)
    Rp_w = np.asarray(Rp_w, dtype=np.float32)
    rel = np.asarray(relation).astype(np.float32)  # values in {0, 1}

    wi = tag_rep[int(tag1_idx)].astype(np.float32)
    wj = tag_rep[int(tag2_idx)].astype(np.float32)

    amax_w = max(np.abs(Lp_w).max(), np.abs(Rp_w).max())
    amax_v = max(np.abs(wi).max(), np.abs(wj).max())
    s_w = f8max * F8_TARGET_FRAC / float(amax_w)
    s_v = f8max * F8_TARGET_FRAC / float(amax_v)
    k2inv = 1.0 / (s_w * s_v) ** 2

    v = (np.concatenate([wi, -wj]) * s_v).astype(np.float32)   # [600]
    v_hi = v.astype(f8np)
    v_lo = (v - v_hi.astype(np.float32)).astype(f8np)

    wv_arr = np.zeros((120, 32), dtype=f8np)
    for half, vq in ((0, v_hi), (1, v_lo)):
        for g in range(3):
            kp = KPHYS[g]
            for i in range(2):
                wv_arr[0:kp, i * 16 + 3 * half + g] = vq[
                    KBASE[g] + i * kp : KBASE[g] + (i + 1) * kp
                ]

    in_maps = []
    for cidx in range(N_CORES):
        sl = slice(cidx * P_LOC, (cidx + 1) * P_LOC)
        # X_T[e, p*300+h] = concat(L, R over e)[p, h, e] * s_w, fp8
        xc = np.concatenate(
            [
                Lp_w[sl].transpose(2, 0, 1),   # [300, 128, 300]
                Rp_w[sl].transpose(2, 0, 1),
            ],
            axis=0,
        ).reshape(KK, P_LOC * H)
        xq = (xc * s_w).astype(f8np)

        core_map = {"wv": wv_arr}
        for g in range(3):
            kp = KPHYS[g]
            blk = xq[KBASE[g] : KBASE[g] + 2 * kp]        # [2*kp, 38400]
            core_map[f"xd{g}"] = np.ascontiguousarray(
                blk.reshape(2, kp, P_LOC * H).transpose(1, 2, 0)
            ).reshape(kp, 2 * P_LOC * H)

        rel_c = rel[sl]
        ra = np.zeros((1, 513), dtype=np.float32)
        ra[0, 0:128] = rel_c * k2inv
        ra[0, 128:256] = (1.0 - rel_c) * k2inv
        ra[0, 256:384] = rel_c
        ra[0, 384:512] = 1.0 - rel_c
        core_map["rm"] = ra
        in_maps.append(core_map)

    kw = {}
    if TRACE:
        kw = dict(trace=True, trace_cores=[0])
    res = run_bass_kernel_spmd(nc, in_maps, core_ids=list(range(N_CORES)), **kw)
    LAST_RESULT = res

    out_full = np.empty((4, P_TOTAL), dtype=np.float32)
    for cidx in range(N_CORES):
        out_full[:, cidx * P_LOC : (cidx + 1) * P_LOC] = res.results[cidx]["out"]
    return out_full


# revision 11
# speedup vs baseline: 1.4964x; 1.4940x over previous
"""Trainium2 Bass kernel for the KnowledgeGraphEmbedding loss.

Computes, for P=1024 relations sharded 128-per-core across 8 NeuronCores:
    li = Lp_w[p] @ wi          (wi = tag_rep[tag1_idx])
    rj = Rp_w[p] @ wj          (wj = tag_rep[tag2_idx])
    dist[p] = sum_h (li - rj)^2
    out = [dist*rel, dist*(1-rel), rel, 1-rel]   (rel in {0,1})

Strategy (memory-bound; ~23MB of fp8 weights streamed per core):
  - Weights quantized to fp8 e3m4 (4x fewer bytes than f32; overall rel
    err ~5.7e-3 vs the 2e-2 gate). Host packs, per core,
    X_T[e, p*300+h] = concat(L, R over e)[p, h, e] * s_w and
    v = [wi; -wj] * s_v.
  - TensorE contracts: per relation, 5 matmuls (K=120 chunks of the
    600-long contraction, N=300 h-columns, stationary v-chunk [120,1])
    accumulate diff[p, 0:300] into one PSUM row at partition 0.
  - DMA: 3 mega-DMAs per 16-relation round (rows 0-239 / 240-479 /
    480-599 of X_T), spread over SP-HWDGE, ACT-HWDGE and gpsimd-SWDGE
    rings to saturate HBM.
  - Drain: one fused square+reduce per relation: ACT activation(Square,
    accum_out) for 9/16, DVE (copy + scalar_tensor_tensor accum) for 7/16.
  - Bins at the end are a few [1,128] DVE ops + one DMA.
"""

from contextlib import ExitStack

import numpy as np

N_CORES = 8
P_TOTAL = 1024
H = 300
E = 300
P_LOC = P_TOTAL // N_CORES   # 128 relations per core
KK = 600                     # contraction length (L and R concatenated)
CHUNK = 120                  # contraction rows per matmul
NCHUNK = KK // CHUNK         # 5
ROUND_P = 16                 # relations per DMA round
NROUND = P_LOC // ROUND_P    # 8
CSUP = ROUND_P * H           # 4800 columns per round

F8_TARGET_FRAC = 0.6         # amax target as fraction of fp8 max

TRACE = False
LAST_RESULT = None

_CACHE: dict = {}


def _build_nc():
    import concourse.bacc as bacc
    import concourse.mybir as mybir
    import concourse.tile as tile

    f32 = mybir.dt.float32
    f8 = mybir.dt.float8e3

    nc = bacc.Bacc("TRN2", debug=False)

    xt = nc.dram_tensor("xt", [KK, P_LOC * H], f8, kind="ExternalInput").ap()
    wv = nc.dram_tensor("wv", [CHUNK, NCHUNK], f8, kind="ExternalInput").ap()
    # rm: [rel*k2inv(128), (1-rel)*k2inv(128), rel(128), 1-rel(128), 0.0]
    rm = nc.dram_tensor("rm", [1, 513], f32, kind="ExternalInput").ap()
    out = nc.dram_tensor("out", [4, P_LOC], f32, kind="ExternalOutput").ap()

    with tile.TileContext(nc) as tc, ExitStack() as ctx:
        const_pool = ctx.enter_context(tc.tile_pool(name="const", bufs=1))
        data_pool = ctx.enter_context(tc.tile_pool(name="data", bufs=2))
        psum_pool = ctx.enter_context(
            tc.tile_pool(name="psum", bufs=8, space="PSUM")
        )
        scr_pool = ctx.enter_context(tc.tile_pool(name="scr", bufs=2))

        v_sb = const_pool.tile([CHUNK, NCHUNK], f8)
        nc.sync.dma_start(v_sb[:], wv[:])
        rm_sb = const_pool.tile([1, 513], f32)
        nc.sync.dma_start(rm_sb[:], rm[:])
        dist_sb = const_pool.tile([1, P_LOC], f32)
        outp = const_pool.tile([1, 4 * P_LOC], f32)

        # DMA groups: (row base, n chunk-rows, queue)
        dma_groups = ((0, 2, nc.sync), (240, 2, nc.scalar), (480, 1, nc.gpsimd))

        for r in range(NROUND):
            csl = slice(r * CSUP, (r + 1) * CSUP)
            dts = []
            for gi, (base, nsub, q) in enumerate(dma_groups):
                dt_ = data_pool.tile([CHUNK, nsub * CSUP], f8, name=f"dt{gi}")
                src = xt[base : base + nsub * CHUNK, csl]
                if nsub > 1:
                    src = src.rearrange("(i k) c -> k i c", i=nsub)
                    dst = dt_.rearrange("k (i c) -> k i c", i=nsub)
                else:
                    dst = dt_[:]
                q.dma_start(dst, src)
                dts.append(dt_)

            for pi in range(ROUND_P):
                pt = psum_pool.tile([1, H], f32, name="pt")
                row = pt[0:1, :]
                for c in range(NCHUNK):
                    gi, sub = (c // 2, c % 2) if c < 4 else (2, 0)
                    off = sub * CSUP + pi * H
                    nc.tensor.matmul(
                        out=row,
                        lhsT=v_sb[:, c : c + 1],
                        rhs=dts[gi][:, off : off + H],
                        start=(c == 0),
                        stop=(c == NCHUNK - 1),
                    )

                p = r * ROUND_P + pi
                col = dist_sb[0:1, p : p + 1]
                if pi % 16 < 9:
                    nc.scalar.activation(
                        row,
                        row,
                        mybir.ActivationFunctionType.Square,
                        bias=rm_sb[0:1, 512:513],
                        scale=1.0,
                        accum_out=col,
                    )
                else:
                    scr = scr_pool.tile([1, H], f32, name="scr")
                    srow = scr[0:1, :]
                    nc.vector.tensor_copy(srow, row)
                    nc.vector.scalar_tensor_tensor(
                        out=srow,
                        in0=srow,
                        scalar=1.0,
                        in1=srow,
                        op0=mybir.AluOpType.mult,
                        op1=mybir.AluOpType.mult,
                        accum_out=col,
                    )

        d = dist_sb[0:1, :]
        nc.vector.tensor_mul(outp[:, 0:128], rm_sb[:, 0:128], d)
        nc.vector.tensor_mul(outp[:, 128:256], rm_sb[:, 128:256], d)
        nc.vector.tensor_copy(outp[:, 256:512], rm_sb[:, 256:512])
        nc.sync.dma_start(
            out[:, :], outp.rearrange("p (b q) -> p b q", b=4)
        )

    nc.compile()
    return nc


def kernel(tag_rep, Lp_w, Rp_w, relation, tag1_idx, tag2_idx):
    global LAST_RESULT
    import ml_dtypes
    from concourse.bass_utils import run_bass_kernel_spmd

    f8np = ml_dtypes.float8_e3m4
    f8max = float(ml_dtypes.finfo(f8np).max)

    if "nc" not in _CACHE:
        _CACHE["nc"] = _build_nc()
    nc = _CACHE["nc"]

    tag_rep = np.asarray(tag_rep)
    Lp_w = np.asarray(Lp_w, dtype=np.float32)
    Rp_w = np.asarray(Rp_w, dtype=np.float32)
    rel = np.asarray(relation).astype(np.float32)  # values in {0, 1}

    wi = tag_rep[int(tag1_idx)].astype(np.float32)
    wj = tag_rep[int(tag2_idx)].astype(np.float32)

    amax_w = max(np.abs(Lp_w).max(), np.abs(Rp_w).max())
    amax_v = max(np.abs(wi).max(), np.abs(wj).max())
    s_w = f8max * F8_TARGET_FRAC / float(amax_w)
    s_v = f8max * F8_TARGET_FRAC / float(amax_v)
    k2inv = 1.0 / (s_w * s_v) ** 2

    v = (np.concatenate([wi, -wj]) * s_v).astype(f8np)         # [600]
    wv_arr = np.ascontiguousarray(v.reshape(NCHUNK, CHUNK).T)  # [120, 5]

    in_maps = []
    for cidx in range(N_CORES):
        sl = slice(cidx * P_LOC, (cidx + 1) * P_LOC)
        # X_T[e, p*300+h] = concat(L, R over e)[p, h, e] * s_w, fp8
        xc = np.concatenate(
            [
                Lp_w[sl].transpose(2, 0, 1),   # [300, 128, 300]
                Rp_w[sl].transpose(2, 0, 1),
            ],
            axis=0,
        ).reshape(KK, P_LOC * H)
        xq = (xc * s_w).astype(f8np)

        rel_c = rel[sl]
        ra = np.zeros((1, 513), dtype=np.float32)
        ra[0, 0:128] = rel_c * k2inv
        ra[0, 128:256] = (1.0 - rel_c) * k2inv
        ra[0, 256:384] = rel_c
        ra[0, 384:512] = 1.0 - rel_c

        in_maps.append({"xt": xq, "wv": wv_arr, "rm": ra})

    kw = {}
    if TRACE:
        kw = dict(trace=True, trace_cores=[0])
    res = run_bass_kernel_spmd(nc, in_maps, core_ids=list(range(N_CORES)), **kw)
    LAST_RESULT = res

    out_full = np.empty((4, P_TOTAL), dtype=np.float32)
    for cidx in range(N_CORES):
        out_full[:, cidx * P_LOC : (cidx + 1) * P_LOC] = res.results[cidx]["out"]
    return out_full


# revision 12
# speedup vs baseline: 1.5064x; 1.0067x over previous
"""Trainium2 Bass kernel for the KnowledgeGraphEmbedding loss.

Computes, for P=1024 relations sharded 128-per-core across 8 NeuronCores:
    li = Lp_w[p] @ wi          (wi = tag_rep[tag1_idx])
    rj = Rp_w[p] @ wj          (wj = tag_rep[tag2_idx])
    dist[p] = sum_h (li - rj)^2
    out = [dist*rel, dist*(1-rel), rel, 1-rel]   (rel in {0,1})

Strategy (memory-bound; ~23MB of fp8 weights streamed per core):
  - Weights quantized to fp8 e3m4 (4x fewer bytes than f32; overall rel
    err ~5.7e-3 vs the 2e-2 gate). Host packs, per core,
    X_T[e, c] = concat(L, R over e)[p, h, e] * s_w with columns ordered
    (round, stream, p-within, h) so each DMA round is one contiguous
    slice, and v = [wi; -wj] * s_v.
  - TensorE contracts with 4-way column tiling (tile_position=(0,32j)):
    four relation-streams run concurrently on the array; per relation,
    5 matmuls (K=120 chunks, N=300 h-columns, stationary v-chunk
    replicated over M_REP columns) accumulate diff[p, :] into a PSUM row
    at partition 32j. M_REP>1 pads the stationary M dim to dodge the
    thin-M PE throttle.
  - DMA: 3 mega-DMAs per 16-relation round (rows 0-239 / 240-479 /
    480-599 of X_T) on the SP-HWDGE, ACT-HWDGE and gpsimd-SWDGE rings.
  - Drain: one fused square+reduce per relation: ACT activation(Square,
    accum_out) for 9/16, DVE (copy + scalar_tensor_tensor accum) for 7/16.
  - Bins at the end are a few [1,32] DVE ops + 4 small DMAs.
"""

from contextlib import ExitStack

import numpy as np

N_CORES = 8
P_TOTAL = 1024
H = 300
E = 300
P_LOC = P_TOTAL // N_CORES   # 128 relations per core
KK = 600                     # contraction length (L and R concatenated)
CHUNK = 120                  # contraction rows per matmul
NCHUNK = KK // CHUNK         # 5
NSTREAM = 4                  # PE column tiles
ROUND_P = 16                 # relations per DMA round (4 per stream)
NROUND = P_LOC // ROUND_P    # 8
RQ = ROUND_P // NSTREAM      # 4 relations per (round, stream)
CSUP = ROUND_P * H           # 4800 columns per round
M_REP = 32                   # stationary columns (v replicated)

F8_TARGET_FRAC = 0.6         # amax target as fraction of fp8 max

TRACE = False
LAST_RESULT = None

_CACHE: dict = {}


def _build_nc():
    import concourse.bacc as bacc
    import concourse.mybir as mybir
    import concourse.tile as tile

    f32 = mybir.dt.float32
    f8 = mybir.dt.float8e3

    nc = bacc.Bacc("TRN2", debug=False)

    xt = nc.dram_tensor("xt", [KK, P_LOC * H], f8, kind="ExternalInput").ap()
    wv = nc.dram_tensor("wv", [CHUNK, NCHUNK * M_REP], f8,
                        kind="ExternalInput").ap()
    # rm row j: [rel_j*k2inv(32), (1-rel_j)*k2inv(32), rel_j(32),
    #            1-rel_j(32), 0.0] -> 129 cols
    rm = nc.dram_tensor("rm", [NSTREAM, 129], f32, kind="ExternalInput").ap()
    out = nc.dram_tensor("out", [4, P_LOC], f32, kind="ExternalOutput").ap()

    with tile.TileContext(nc) as tc, ExitStack() as ctx:
        const_pool = ctx.enter_context(tc.tile_pool(name="const", bufs=1))
        data_pool = ctx.enter_context(tc.tile_pool(name="data", bufs=2))
        psum_pool = ctx.enter_context(
            tc.tile_pool(name="psum", bufs=2, space="PSUM")
        )
        scr_pool = ctx.enter_context(tc.tile_pool(name="scr", bufs=2))

        v_sb = const_pool.tile([CHUNK, NCHUNK * M_REP], f8)
        nc.sync.dma_start(v_sb[:], wv[:])
        rm_sb = const_pool.tile([P_LOC, 129], f32)
        for j in range(NSTREAM):
            nc.sync.dma_start(rm_sb[32 * j : 32 * j + 1, :], rm[j : j + 1, :])
        dist_sb = const_pool.tile([P_LOC, 32], f32)
        outp = const_pool.tile([P_LOC, 128], f32)

        # DMA groups: (row base, n chunk-rows, queue)
        dma_groups = ((0, 2, nc.sync), (240, 2, nc.scalar), (480, 1, nc.gpsimd))

        for r in range(NROUND):
            csl = slice(r * CSUP, (r + 1) * CSUP)
            dts = []
            for gi, (base, nsub, q) in enumerate(dma_groups):
                dt_ = data_pool.tile([CHUNK, nsub * CSUP], f8, name=f"dt{gi}")
                src = xt[base : base + nsub * CHUNK, csl]
                if nsub > 1:
                    src = src.rearrange("(i k) c -> k i c", i=nsub)
                    dst = dt_.rearrange("k (i c) -> k i c", i=nsub)
                else:
                    dst = dt_[:]
                q.dma_start(dst, src)
                dts.append(dt_)

            for q_ in range(RQ):
                pts = [psum_pool.tile([P_LOC, H], f32, name=f"pt{jj}")
                       for jj in range(NSTREAM)]
                for c in range(NCHUNK):
                    gi, sub = (c // 2, c % 2) if c < 4 else (2, 0)
                    for j in range(NSTREAM):
                        off = sub * CSUP + j * (RQ * H) + q_ * H
                        nc.tensor.matmul(
                            out=pts[j][32 * j : 32 * j + M_REP, :],
                            lhsT=v_sb[:, c * M_REP : (c + 1) * M_REP],
                            rhs=dts[gi][:, off : off + H],
                            start=(c == 0),
                            stop=(c == NCHUNK - 1),
                            tile_position=(0, 32 * j),
                        )

                for j in range(NSTREAM):
                    row = pts[j][32 * j : 32 * j + 1, :]
                    m = r * RQ + q_     # p = 32*j + m
                    col = dist_sb[32 * j : 32 * j + 1, m : m + 1]
                    if (r * RQ * NSTREAM + q_ * NSTREAM + j) % 16 < 9:
                        nc.scalar.activation(
                            row,
                            row,
                            mybir.ActivationFunctionType.Square,
                            bias=rm_sb[32 * j : 32 * j + 1, 128:129],
                            scale=1.0,
                            accum_out=col,
                        )
                    else:
                        scr = scr_pool.tile([P_LOC, H], f32, name="scr")
                        srow = scr[32 * j : 32 * j + 1, :]
                        nc.vector.tensor_copy(srow, row)
                        nc.vector.scalar_tensor_tensor(
                            out=srow,
                            in0=srow,
                            scalar=1.0,
                            in1=srow,
                            op0=mybir.AluOpType.mult,
                            op1=mybir.AluOpType.mult,
                            accum_out=col,
                        )

        for j in range(NSTREAM):
            r32 = slice(32 * j, 32 * j + 1)
            d = dist_sb[r32, :]
            o = outp[r32, :]
            nc.vector.tensor_mul(o[:, 0:32], rm_sb[r32, 0:32], d)
            nc.vector.tensor_mul(o[:, 32:64], rm_sb[r32, 32:64], d)
            nc.vector.tensor_copy(o[:, 64:128], rm_sb[r32, 64:128])
            nc.sync.dma_start(
                out[:, 32 * j : 32 * j + 32],
                o.rearrange("p (b q) -> p b q", b=4),
            )

    nc.compile()
    return nc


def kernel(tag_rep, Lp_w, Rp_w, relation, tag1_idx, tag2_idx):
    global LAST_RESULT
    import ml_dtypes
    from concourse.bass_utils import run_bass_kernel_spmd

    f8np = ml_dtypes.float8_e3m4
    f8max = float(ml_dtypes.finfo(f8np).max)

    if "nc" not in _CACHE:
        _CACHE["nc"] = _build_nc()
    nc = _CACHE["nc"]

    tag_rep = np.asarray(tag_rep)
    Lp_w = np.asarray(Lp_w, dtype=np.float32)
    Rp_w = np.asarray(Rp_w, dtype=np.float32)
    rel = np.asarray(relation).astype(np.float32)  # values in {0, 1}

    wi = tag_rep[int(tag1_idx)].astype(np.float32)
    wj = tag_rep[int(tag2_idx)].astype(np.float32)

    amax_w = max(np.abs(Lp_w).max(), np.abs(Rp_w).max())
    amax_v = max(np.abs(wi).max(), np.abs(wj).max())
    s_w = f8max * F8_TARGET_FRAC / float(amax_w)
    s_v = f8max * F8_TARGET_FRAC / float(amax_v)
    k2inv = 1.0 / (s_w * s_v) ** 2

    v = (np.concatenate([wi, -wj]) * s_v).astype(f8np)  # [600]
    vc = np.ascontiguousarray(v.reshape(NCHUNK, CHUNK).T)  # [120, 5]
    wv_arr = np.ascontiguousarray(
        np.repeat(vc[:, :, None], M_REP, axis=2).reshape(CHUNK, NCHUNK * M_REP)
    )

    # column order: p-blocks sequenced (round, stream, q)
    p_order = np.array(
        [32 * j + RQ * r + q
         for r in range(NROUND) for j in range(NSTREAM) for q in range(RQ)],
        dtype=np.int64,
    )

    in_maps = []
    for cidx in range(N_CORES):
        sl = slice(cidx * P_LOC, (cidx + 1) * P_LOC)
        xc = np.concatenate(
            [
                Lp_w[sl].transpose(2, 0, 1),   # [300, 128, 300]
                Rp_w[sl].transpose(2, 0, 1),
            ],
            axis=0,
        )
        xq = (xc[:, p_order, :] * s_w).astype(f8np).reshape(KK, P_LOC * H)

        rel_c = rel[sl]
        rm_arr = np.zeros((NSTREAM, 129), dtype=np.float32)
        for j in range(NSTREAM):
            rj = rel_c[32 * j : 32 * j + 32]
            rm_arr[j, 0:32] = rj * k2inv
            rm_arr[j, 32:64] = (1.0 - rj) * k2inv
            rm_arr[j, 64:96] = rj
            rm_arr[j, 96:128] = 1.0 - rj

        in_maps.append({"xt": xq, "wv": wv_arr, "rm": rm_arr})

    kw = {}
    if TRACE:
        kw = dict(trace=True, trace_cores=[0])
    res = run_bass_kernel_spmd(nc, in_maps, core_ids=list(range(N_CORES)), **kw)
    LAST_RESULT = res

    out_full = np.empty((4, P_TOTAL), dtype=np.float32)
    for cidx in range(N_CORES):
        out_full[:, cidx * P_LOC : (cidx + 1) * P_LOC] = res.results[cidx]["out"]
    return out_full


# revision 13
# speedup vs baseline: 1.5737x; 1.0447x over previous
"""Trainium2 Bass kernel for the KnowledgeGraphEmbedding loss.

Computes, for P=1024 relations sharded 128-per-core across 8 NeuronCores:
    li = Lp_w[p] @ wi          (wi = tag_rep[tag1_idx])
    rj = Rp_w[p] @ wj          (wj = tag_rep[tag2_idx])
    dist[p] = sum_h (li - rj)^2
    out = [dist*rel, dist*(1-rel), rel, 1-rel]   (rel in {0,1})

Strategy (memory-bound; ~23MB of fp8 weights streamed per core):
  - Weights quantized to fp8 e3m4 (4x fewer bytes than f32; overall rel
    err ~5.7e-3 vs the 2e-2 gate). Host packs, per core,
    X_T[e, c] = concat(L, R over e)[p, h, e] * s_w with columns ordered
    (round, stream, p-within, h) so each DMA round is one contiguous
    slice, and v = [wi; -wj] * s_v.
  - TensorE contracts with 4-way column tiling (tile_position=(0,32j)):
    four relation-streams run concurrently on the array; per relation,
    5 matmuls (K=120 chunks, N=300 h-columns, stationary v-chunk
    replicated over M_REP columns) accumulate diff[p, :] into a PSUM row
    at partition 32j. M_REP>1 pads the stationary M dim to dodge the
    thin-M PE throttle.
  - DMA: 3 mega-DMAs per 16-relation round (rows 0-239 / 240-479 /
    480-599 of X_T) on the SP-HWDGE, ACT-HWDGE and gpsimd-SWDGE rings.
  - Drain: one fused square+reduce per relation: ACT activation(Square,
    accum_out) for 9/16, DVE (copy + scalar_tensor_tensor accum) for 7/16.
  - Bins at the end are a few [1,32] DVE ops + 4 small DMAs.
"""

from contextlib import ExitStack

import numpy as np

N_CORES = 8
P_TOTAL = 1024
H = 300
E = 300
P_LOC = P_TOTAL // N_CORES   # 128 relations per core
KK = 600                     # contraction length (L and R concatenated)
CHUNK = 120                  # contraction rows per matmul
NCHUNK = KK // CHUNK         # 5
NSTREAM = 4                  # PE column tiles
ROUND_P = 16                 # relations per DMA round (4 per stream)
NROUND = P_LOC // ROUND_P    # 8
RQ = ROUND_P // NSTREAM      # 4 relations per (round, stream)
CSUP = ROUND_P * H           # 4800 columns per round
M_REP = 1                    # stationary columns (v replicated)

F8_TARGET_FRAC = 0.6         # amax target as fraction of fp8 max

TRACE = False
LAST_RESULT = None

_CACHE: dict = {}


def _build_nc():
    import concourse.bacc as bacc
    import concourse.mybir as mybir
    import concourse.tile as tile

    f32 = mybir.dt.float32
    f8 = mybir.dt.float8e3

    nc = bacc.Bacc("TRN2", debug=False)

    xt = nc.dram_tensor("xt", [KK, P_LOC * H], f8, kind="ExternalInput").ap()
    wv = nc.dram_tensor("wv", [CHUNK, NCHUNK * M_REP], f8,
                        kind="ExternalInput").ap()
    # rm row j: [rel_j*k2inv(32), (1-rel_j)*k2inv(32), rel_j(32),
    #            1-rel_j(32), 0.0] -> 129 cols
    rm = nc.dram_tensor("rm", [NSTREAM, 129], f32, kind="ExternalInput").ap()
    out = nc.dram_tensor("out", [4, P_LOC], f32, kind="ExternalOutput").ap()

    with tile.TileContext(nc) as tc, ExitStack() as ctx:
        const_pool = ctx.enter_context(tc.tile_pool(name="const", bufs=1))
        data_pool = ctx.enter_context(tc.tile_pool(name="data", bufs=2))
        psum_pool = ctx.enter_context(
            tc.tile_pool(name="psum", bufs=2, space="PSUM")
        )
        scr_pool = ctx.enter_context(tc.tile_pool(name="scr", bufs=2))

        v_sb = const_pool.tile([CHUNK, NCHUNK * M_REP], f8)
        nc.sync.dma_start(v_sb[:], wv[:])
        rm_sb = const_pool.tile([P_LOC, 129], f32)
        for j in range(NSTREAM):
            nc.sync.dma_start(rm_sb[32 * j : 32 * j + 1, :], rm[j : j + 1, :])
        dist_sb = const_pool.tile([P_LOC, 32], f32)
        outp = const_pool.tile([P_LOC, 128], f32)

        # DMA groups: (row base, n chunk-rows, queue)
        dma_groups = ((0, 2, nc.sync), (240, 2, nc.scalar), (480, 1, nc.gpsimd))

        for r in range(NROUND):
            csl = slice(r * CSUP, (r + 1) * CSUP)
            dts = []
            for gi, (base, nsub, q) in enumerate(dma_groups):
                dt_ = data_pool.tile([CHUNK, nsub * CSUP], f8, name=f"dt{gi}")
                src = xt[base : base + nsub * CHUNK, csl]
                if nsub > 1:
                    src = src.rearrange("(i k) c -> k i c", i=nsub)
                    dst = dt_.rearrange("k (i c) -> k i c", i=nsub)
                else:
                    dst = dt_[:]
                q.dma_start(dst, src)
                dts.append(dt_)

            for q_ in range(RQ):
                pts = [psum_pool.tile([P_LOC, H], f32, name=f"pt{jj}")
                       for jj in range(NSTREAM)]
                for j in range(NSTREAM):
                    for c in range(NCHUNK):
                        gi, sub = (c // 2, c % 2) if c < 4 else (2, 0)
                        off = sub * CSUP + j * (RQ * H) + q_ * H
                        nc.tensor.matmul(
                            out=pts[j][32 * j : 32 * j + M_REP, :],
                            lhsT=v_sb[:, c * M_REP : (c + 1) * M_REP],
                            rhs=dts[gi][:, off : off + H],
                            start=(c == 0),
                            stop=(c == NCHUNK - 1),
                            tile_position=(0, 32 * j),
                        )

                for j in range(NSTREAM):
                    row = pts[j][32 * j : 32 * j + 1, :]
                    m = r * RQ + q_     # p = 32*j + m
                    col = dist_sb[32 * j : 32 * j + 1, m : m + 1]
                    if (r * RQ * NSTREAM + q_ * NSTREAM + j) % 16 < 9:
                        nc.scalar.activation(
                            row,
                            row,
                            mybir.ActivationFunctionType.Square,
                            bias=rm_sb[32 * j : 32 * j + 1, 128:129],
                            scale=1.0,
                            accum_out=col,
                        )
                    else:
                        scr = scr_pool.tile([P_LOC, H], f32, name="scr")
                        srow = scr[32 * j : 32 * j + 1, :]
                        nc.vector.tensor_copy(srow, row)
                        nc.vector.scalar_tensor_tensor(
                            out=srow,
                            in0=srow,
                            scalar=1.0,
                            in1=srow,
                            op0=mybir.AluOpType.mult,
                            op1=mybir.AluOpType.mult,
                            accum_out=col,
                        )

        for j in range(NSTREAM):
            r32 = slice(32 * j, 32 * j + 1)
            d = dist_sb[r32, :]
            o = outp[r32, :]
            nc.vector.tensor_mul(o[:, 0:32], rm_sb[r32, 0:32], d)
            nc.vector.tensor_mul(o[:, 32:64], rm_sb[r32, 32:64], d)
            nc.vector.tensor_copy(o[:, 64:128], rm_sb[r32, 64:128])
            nc.sync.dma_start(
                out[:, 32 * j : 32 * j + 32],
                o.rearrange("p (b q) -> p b q", b=4),
            )

    nc.compile()
    return nc


def kernel(tag_rep, Lp_w, Rp_w, relation, tag1_idx, tag2_idx):
    global LAST_RESULT
    import ml_dtypes
    from concourse.bass_utils import run_bass_kernel_spmd

    f8np = ml_dtypes.float8_e3m4
    f8max = float(ml_dtypes.finfo(f8np).max)

    if "nc" not in _CACHE:
        _CACHE["nc"] = _build_nc()
    nc = _CACHE["nc"]

    tag_rep = np.asarray(tag_rep)
    Lp_w = np.asarray(Lp_w, dtype=np.float32)
    Rp_w = np.asarray(Rp_w, dtype=np.float32)
    rel = np.asarray(relation).astype(np.float32)  # values in {0, 1}

    wi = tag_rep[int(tag1_idx)].astype(np.float32)
    wj = tag_rep[int(tag2_idx)].astype(np.float32)

    amax_w = max(np.abs(Lp_w).max(), np.abs(Rp_w).max())
    amax_v = max(np.abs(wi).max(), np.abs(wj).max())
    s_w = f8max * F8_TARGET_FRAC / float(amax_w)
    s_v = f8max * F8_TARGET_FRAC / float(amax_v)
    k2inv = 1.0 / (s_w * s_v) ** 2

    v = (np.concatenate([wi, -wj]) * s_v).astype(f8np)  # [600]
    vc = np.ascontiguousarray(v.reshape(NCHUNK, CHUNK).T)  # [120, 5]
    wv_arr = np.ascontiguousarray(
        np.repeat(vc[:, :, None], M_REP, axis=2).reshape(CHUNK, NCHUNK * M_REP)
    )

    # column order: p-blocks sequenced (round, stream, q)
    p_order = np.array(
        [32 * j + RQ * r + q
         for r in range(NROUND) for j in range(NSTREAM) for q in range(RQ)],
        dtype=np.int64,
    )

    in_maps = []
    for cidx in range(N_CORES):
        sl = slice(cidx * P_LOC, (cidx + 1) * P_LOC)
        xc = np.concatenate(
            [
                Lp_w[sl].transpose(2, 0, 1),   # [300, 128, 300]
                Rp_w[sl].transpose(2, 0, 1),
            ],
            axis=0,
        )
        xq = (xc[:, p_order, :] * s_w).astype(f8np).reshape(KK, P_LOC * H)

        rel_c = rel[sl]
        rm_arr = np.zeros((NSTREAM, 129), dtype=np.float32)
        for j in range(NSTREAM):
            rj = rel_c[32 * j : 32 * j + 32]
            rm_arr[j, 0:32] = rj * k2inv
            rm_arr[j, 32:64] = (1.0 - rj) * k2inv
            rm_arr[j, 64:96] = rj
            rm_arr[j, 96:128] = 1.0 - rj

        in_maps.append({"xt": xq, "wv": wv_arr, "rm": rm_arr})

    kw = {}
    if TRACE:
        kw = dict(trace=True, trace_cores=[0])
    res = run_bass_kernel_spmd(nc, in_maps, core_ids=list(range(N_CORES)), **kw)
    LAST_RESULT = res

    out_full = np.empty((4, P_TOTAL), dtype=np.float32)
    for cidx in range(N_CORES):
        out_full[:, cidx * P_LOC : (cidx + 1) * P_LOC] = res.results[cidx]["out"]
    return out_full
